# revision 33
# baseline (speedup 1.0000x reference)
"""Trainium2 Bass kernel for nn_Decoder_63505386438998.

6-layer post-norm transformer decoder (self-attn + cross-attn + FFN),
B=16, S=256, D=768, H=12, DFF=2048, fp32 in/out.

Strategy: pure data parallel — 8 cores x 2 batches each, weights
replicated, no collectives. Feature-major [feature, token] layout so
every linear is a chain of 128x128xN matmuls with weights stationary.

v2 over the baseline:
- Weights are host-repacked to [128, k*cols] so each matrix loads with
  ONE large contiguous DMA (1-3 MB) instead of dozens of 64 KB strided
  slices; weight tiles are SBUF-resident, slot-recycled by tag.
- Attention runs as per-(head-pair, batch) chains: one fused exp over
  the whole score strip, one combined causal-mask multiply, one shared
  reciprocal per pair, and the out-projection interleaved (lagged one
  pair) into 3 packed PSUM banks. This keeps the tensor engine dense
  through softmax so the HAM clock stays at 2.4 GHz.
- Cross-attn K/V projections are emitted as fill work under LN1's
  serial chain; weight prefetches are emitted as soon as the previous
  tenant of their SBUF slot is dead.
"""

import sys

import numpy as np

try:
    import concourse.bass as bass
except ImportError:  # toolchain lives here in the execution container
    sys.path.insert(0, "/opt/trn_rl_repo")
    import concourse.bass as bass

import concourse.bacc as bacc
import concourse.mybir as mybir
from concourse import tile
from concourse.bass_utils import run_bass_kernel_spmd

P = 128
NB = 2            # batches per core
SB = 256          # sequence length
T = NB * SB       # tokens per core = 512
D = 768
KD = D // P       # 6 feature tiles
DFF = 2048
KF = DFF // P     # 16 hidden tiles
H = 12
HD = 64
NP = H // 2       # 6 head pairs
L = 6
NCORES = 8
EPS = 1e-5

F32 = mybir.dt.float32
BF = mybir.dt.bfloat16
AF = mybir.ActivationFunctionType
ALU = mybir.AluOpType

# packed per-layer vector columns (layout [L, 128, NV]); value = col base
VC_BQ, VC_BK, VC_CBQ, VC_CBK, VC_CO, VC_CCO = 0, 6, 12, 18, 24, 30
VC_B2, VC_L1G, VC_L1B, VC_L2G, VC_L2B = 36, 42, 48, 54, 60
VC_L3G, VC_L3B, VC_B1 = 66, 72, 78
NV = 78 + KF  # 94


def build_nc(nl=L):
    nc = bacc.Bacc(None, target_bir_lowering=False)

    xpeB_d = nc.declare_dram_parameter("xpeB", [D, T], BF, False)
    memB_d = nc.declare_dram_parameter("memB", [D, T], BF, False)
    # weights repacked host-side: [nl, 128, KD*D] etc (k-tiles adjacent)
    wq_d = nc.declare_dram_parameter("wq", [nl, P, KD * D], BF, False)
    wk_d = nc.declare_dram_parameter("wk", [nl, P, KD * D], BF, False)
    wv_d = nc.declare_dram_parameter("wv", [nl, P, KD * D], BF, False)
    wo_d = nc.declare_dram_parameter("wo", [nl, P, KD * D], BF, False)
    cq_d = nc.declare_dram_parameter("cq", [nl, P, KD * D], BF, False)
    ck_d = nc.declare_dram_parameter("ck", [nl, P, KD * D], BF, False)
    cv_d = nc.declare_dram_parameter("cv", [nl, P, KD * D], BF, False)
    co_d = nc.declare_dram_parameter("co", [nl, P, KD * D], BF, False)
    w1_d = nc.declare_dram_parameter("w1", [nl, P, KD * DFF], BF, False)
    w2_d = nc.declare_dram_parameter("w2", [nl, P, KF * D], BF, False)
    wp_d = nc.declare_dram_parameter("wp", [1, P, KD * D], BF, False)
    vec_d = nc.declare_dram_parameter("vec", [nl, P, NV], F32, False)
    bp_d = nc.declare_dram_parameter("bp", [P, KD + 1], F32, False)
    ones_d = nc.declare_dram_parameter("ones", [P, 4 * P], BF, False)
    mask_d = nc.declare_dram_parameter("maskT", [P, 2 * (SB + P)], BF,
                                       False)
    out_d = nc.declare_dram_parameter("outT", [D, T], F32, True)

    with tile.TileContext(nc) as tc:
        with (
            tc.tile_pool(name="res", bufs=1) as res,
            tc.tile_pool(name="wpool", bufs=1) as wpool,
            tc.tile_pool(name="tmp", bufs=2) as tmp,
            tc.tile_pool(name="attn", bufs=6) as attn,
            tc.tile_pool(name="pp", bufs=3, space="PSUM") as pp,
            tc.tile_pool(name="patt", bufs=1, space="PSUM") as patt,
        ):
            # ---------------- resident tiles ----------------
            xBs = [res.tile([P, T], BF, tag=f"xB{i}", name=f"xB{i}")
                   for i in range(KD)]
            memBs = [res.tile([P, T], BF, tag=f"memB{i}", name=f"memB{i}")
                     for i in range(KD)]
            qTs = [res.tile([P, T], BF, tag=f"qT{i}", name=f"qT{i}")
                   for i in range(KD)]
            kTs = [res.tile([P, T], BF, tag=f"kT{i}", name=f"kT{i}")
                   for i in range(KD)]
            vs = [res.tile([P, D], BF, tag=f"v{i}", name=f"v{i}")
                  for i in range(NB * 2)]
            ckTs = [res.tile([P, T], BF, tag=f"ckT{i}", name=f"ckT{i}")
                    for i in range(KD)]
            cvs = [res.tile([P, D], BF, tag=f"cv{i}", name=f"cv{i}")
                   for i in range(NB * 2)]
            aTs = [res.tile([P, T], BF, tag=f"aT{i}", name=f"aT{i}")
                   for i in range(KD)]
            hTs = [res.tile([P, T], BF, tag=f"hT{i}", name=f"hT{i}")
                   for i in range(KF)]
            vec_sb = res.tile([P, nl * NV], F32, tag="vec", name="vec")
            maskc = res.tile([P, 2 * (SB + P)], BF, tag="maskc",
                             name="maskc")
            bp_sb = res.tile([P, KD + 1], F32, tag="bp", name="bp")
            dumt = res.tile([P, 1], F32, tag="dumt", name="dumt")
            ones3 = res.tile([P, 4 * P], BF, tag="ones", name="ones")
            ones = ones3[:, 0:P]
            zeros = ones3[:, P:2 * P]
            oD = ones3[:, 2 * P:3 * P]  # ones / D for LN stats
            ident = ones3[:, 3 * P:4 * P]

            nc.sync.dma_start(ones3[:], ones_d[:])
            eps_sb = bp_sb[:, KD:KD + 1]
            for i in range(KD):
                nc.sync.dma_start(xBs[i][:], xpeB_d[i * P:(i + 1) * P, :])
                nc.sync.dma_start(memBs[i][:], memB_d[i * P:(i + 1) * P, :])
            nc.sync.dma_start(maskc[:], mask_d[:])
            # warm the exp table set before the first attention
            nc.scalar.activation(dumt[:], ones3[:, 0:1], AF.Exp)
            for l in range(nl):
                nc.sync.dma_start(vec_sb[:, l * NV:(l + 1) * NV], vec_d[l])
            nc.sync.dma_start(bp_sb[:], bp_d[:])

            def vcol(l, base, i):
                return vec_sb[:, l * NV + base + i:l * NV + base + i + 1]

            # ---- weight staging: one contiguous DMA per matrix ----
            def wload(wdram, l, tag, width=KD * D):
                wt = wpool.tile([P, width], BF, tag=tag, name=tag)
                nc.sync.dma_start(wt[:], wdram[l])
                return wt

            # ------------- building blocks -------------
            def proj_fm_blocks(wt, src, nout, out_cb, cw=D,
                               alt_psum_obs=()):
                """thunk per o-block of 2 psum tiles; emitting a thunk
                lays down its matmuls + consume ops. Blocks listed in
                alt_psum_obs borrow the attention sm-slots in PSUM so
                they can run while the po banks are still draining."""
                nko = len(src)

                def blk(ob):
                    o0 = 2 * ob
                    nt = min(2, nout - o0)
                    if ob in alt_psum_obs:
                        pss = [patt.tile([P, T], F32, tag="sm", bufs=2,
                                         name=f"fp{o0 + j}")
                               for j in range(nt)]
                    else:
                        pss = [pp.tile([P, T], F32, tag="pp",
                                       name=f"pp{o0 + j}")
                               for j in range(nt)]
                    for k in range(nko):
                        for j in range(nt):
                            o = o0 + j
                            nc.tensor.matmul(
                                pss[j][:],
                                wt[:, k * cw + o * P:k * cw + (o + 1) * P],
                                src[k][:],
                                start=(k == 0),
                                stop=(k == nko - 1))
                    for j in range(nt):
                        out_cb(o0 + j, pss[j])

                return [lambda ob=ob: blk(ob)
                        for ob in range((nout + 1) // 2)]

            def proj_fm(wt, src, nout, out_cb, cw=D):
                """out[o,t] = sum_k w[:,k*cw+o*128].T @ src[k]"""
                for th in proj_fm_blocks(wt, src, nout, out_cb, cw):
                    th()

            def proj_v_blocks(wt, src, dst, chunks=(0, 1),
                              alt_psum_t0s=()):
                """dst[bt][t, o] (token-major [128, D] tiles) from
                feature-major src; weights are the moving operand.
                One thunk per (chunk, token-pair)."""
                def blk(c0, csz, t0):
                    if c0 == 0 and t0 in alt_psum_t0s:
                        pss = [patt.tile([P, csz], F32, tag="sm", bufs=2,
                                         name=f"fv{t0 + i}")
                               for i in range(2)]
                    else:
                        pss = [pp.tile([P, csz], F32, tag="pp",
                                       name=f"vps{t0 + i}")
                               for i in range(2)]
                    for k in range(KD):
                        for i in range(2):
                            t = t0 + i
                            nc.tensor.matmul(
                                pss[i][:],
                                src[k][:, t * P:(t + 1) * P],
                                wt[:, k * D + c0:k * D + c0 + csz],
                                start=(k == 0), stop=(k == KD - 1))
                    for i in range(2):
                        nc.scalar.copy(dst[t0 + i][:, c0:c0 + csz],
                                       pss[i][:])

                thunks = []
                for ci, c0 in enumerate(range(0, D, 512)):
                    if ci not in chunks:
                        continue
                    csz = min(512, D - c0)
                    for t0 in range(0, 4, 2):
                        thunks.append(
                            lambda c0=c0, csz=csz, t0=t0: blk(c0, csz, t0))
                return thunks

            def proj_v(wt, src, dst, chunks=(0, 1)):
                for th in proj_v_blocks(wt, src, dst, chunks):
                    th()

            def attn_projs(l, wq_t, wk_t, wv_t, bq_base, bk_base,
                           qsrc, kvsrc):
                def kcb(o, ps):
                    nc.vector.tensor_scalar_add(kTs[o][:], ps[:],
                                                vcol(l, bk_base, o))
                def qcb(o, ps):
                    nc.vector.tensor_scalar_add(qTs[o][:], ps[:],
                                                vcol(l, bq_base, o))
                proj_fm(wk_t, kvsrc, KD, kcb)
                proj_v(wv_t, kvsrc, vs)
                proj_fm(wq_t, qsrc, KD, qcb)

            def attn_chains(l, wo_t, co_base, causal, kt, vt):
                """softmax chains per (head-pair, batch) with the
                out-projection interleaved, lagged one pair behind.

                The first out-projection writer of each po region uses
                start=True (zeroing the bank in-place), the last uses
                stop=True; the residual + bias add is folded into the
                consume via scalar_tensor_tensor."""
                W = SB + P if causal else 2 * SB  # score strip width
                for b in range(NB):
                    bq = slice(b * SB, (b + 1) * SB)
                    po = [pp.tile([P, T], F32, tag="pp", name=f"po{m}")
                          for m in range(3)]
                    for m in range(3):
                        nc.tensor.matmul(po[m][:], zeros, memBs[0][:],
                                         start=True, stop=False)

                    def outproj(j):
                        for o in range(KD):
                            m, half = o // 2, o % 2
                            nc.tensor.matmul(
                                po[m][:, half * SB:(half + 1) * SB],
                                wo_t[:, j * D + o * P:j * D + (o + 1) * P],
                                aTs[j][:, bq],
                                start=False, stop=(j == NP - 1))

                    for j in range(NP):
                        at2 = attn.tile([P, 4 * SB], BF, tag="at",
                                        name="at2")
                        for hh in range(2):  # even/odd head of pair j
                            off = 64 * hh
                            hsl = slice(off, off + 64)
                            sc = patt.tile([P, W], F32, tag="sc", bufs=2,
                                           name=f"sc{hh}")
                            for s in range(2):
                                lo = P if (causal and s == 1) else 0
                                w_q = SB - lo
                                ks = kt[j][hsl, b * SB + s * P:
                                           b * SB + (s + 1) * P]
                                qs = qTs[j][hsl, b * SB + lo:(b + 1) * SB]
                                nc.tensor.matmul(sc[:, s * SB:s * SB + w_q],
                                                 ks, qs,
                                                 tile_position=(off, 0))
                            nc.scalar.activation(
                                at2[:, hh * W:(hh + 1) * W], sc[:], AF.Exp)
                        if causal:
                            nc.vector.tensor_tensor(
                                at2[:, 0:2 * W], at2[:, 0:2 * W],
                                maskc[:], ALU.mult)
                        sm = patt.tile([P, 2 * SB], F32, tag="sm", bufs=2,
                                       name="sm")
                        for hh in range(2):
                            c0 = hh * SB
                            for s in range(2):
                                lo = P if (causal and s == 1) else 0
                                nc.tensor.matmul(
                                    sm[:, c0 + lo:c0 + SB], ones[:],
                                    at2[:, hh * W + s * SB:
                                         hh * W + (s + 1) * SB - lo],
                                    start=(hh == 0 and s == 0),
                                    stop=(s == 1))
                        rr = attn.tile([P, 2 * SB], F32, tag="rr", bufs=3,
                                       name="rr")
                        nc.vector.reciprocal_approx_fast(rr[:], sm[:])
                        ao = patt.tile([P, 2 * SB], F32, tag="ao", bufs=1,
                                       name="ao")
                        for hh in range(2):
                            if hh == 0:
                                c0, c1, osl = j * P, j * P + 64, slice(0, 64)
                            else:
                                c0, c1, osl = j * P, (j + 1) * P, slice(0, P)
                            for s in range(2):
                                lo = P if (causal and s == 1) else 0
                                nc.tensor.matmul(
                                    ao[osl, hh * SB + lo:(hh + 1) * SB],
                                    vt[b * 2 + s][:, c0:c1],
                                    at2[:, hh * W + s * SB:
                                         hh * W + (s + 1) * SB - lo],
                                    start=(s == 0), stop=(s == 1))
                        nc.vector.tensor_tensor(
                            aTs[j][0:64, bq], ao[0:64, 0:SB],
                            rr[0:64, 0:SB], ALU.mult)
                        nc.vector.tensor_tensor(
                            aTs[j][64:P, bq], ao[64:P, SB:2 * SB],
                            rr[64:P, SB:2 * SB], ALU.mult)
                        if j > 0:
                            outproj(j - 1)
                    outproj(NP - 1)
                    # consume: residual + folded bias in one DVE op
                    for o in range(KD):
                        m, half = o // 2, o % 2
                        ps = po[m][:, half * SB:(half + 1) * SB]
                        nc.vector.scalar_tensor_tensor(
                            xBs[o][:, bq], ps, vcol(l, co_base, o),
                            xBs[o][:, bq], ALU.add, ALU.add)

            def layernorm(l, g_base, b_base, fill_pre=(), fill=(),
                          hi=False):
                # PE cover for the preceding consume boundary
                for th in fill_pre:
                    th()
                # stats via ones/D stationary: mu and E[x^2] directly
                mu = pp.tile([P, T], F32, tag="pp", name="mu")
                sq_ps = pp.tile([P, T], F32, tag="pp", name="sq_ps")
                for k in range(KD):
                    sq = tmp.tile([P, T], BF, tag="sq", name="sq")
                    nc.vector.tensor_tensor(sq[:], xBs[k][:], xBs[k][:],
                                            ALU.mult)
                    nc.tensor.matmul(mu[:], oD[:], xBs[k][:],
                                     start=(k == 0), stop=(k == KD - 1))
                    nc.tensor.matmul(sq_ps[:], oD[:], sq[:],
                                     start=(k == 0), stop=(k == KD - 1))
                # mu copy on scalar (off the DVE chain); sd straight
                # from PSUM on scalar in parallel. mu/inv in bf16: the
                # per-token scale error cancels at the next LN, so only
                # the network's last LN needs f32 (hi=True).
                sdt = F32 if hi else BF
                mu_sb = tmp.tile([P, T], sdt, tag="mu_sb", bufs=1,
                                 name="mu_sb")
                nc.scalar.copy(mu_sb[:], mu[:])
                sd = tmp.tile([P, T], F32, tag="sd", bufs=1, name="sd")
                nc.scalar.square(sd[:], mu[:])
                m2 = tmp.tile([P, T], F32, tag="m2", bufs=1, name="m2")
                nc.vector.tensor_tensor(m2[:], sq_ps[:], sd[:], ALU.subtract)
                for th in fill:
                    th()  # independent PE work under the serial chain
                # inv = exp(-0.5*ln(var+eps)): ln+exp share one ACT
                # table set, so no table switches against attention exp
                lnv = tmp.tile([P, T], F32, tag="lnv", bufs=1, name="lnv")
                nc.scalar.activation(lnv[:], m2[:], AF.Ln, bias=eps_sb)
                inv = tmp.tile([P, T], sdt, tag="inv", bufs=1, name="inv")
                nc.scalar.activation(inv[:], lnv[:], AF.Exp, scale=-0.5)
                for k in range(KD):
                    sub = tmp.tile([P, T], BF, tag="sub", bufs=3,
                                   name="sub")
                    nc.vector.tensor_tensor(sub[:], xBs[k][:], mu_sb[:],
                                            ALU.subtract)
                    t = tmp.tile([P, T], BF, tag="t", bufs=3, name="t")
                    nc.vector.tensor_tensor(t[:], sub[:], inv[:], ALU.mult)
                    if k % 2 == 0:
                        nc.vector.tensor_scalar(
                            xBs[k][:], t[:], vcol(l, g_base, k),
                            vcol(l, b_base, k), ALU.mult, ALU.add)
                    else:
                        nc.scalar.activation(xBs[k][:], t[:], AF.Identity,
                                             bias=vcol(l, b_base, k),
                                             scale=vcol(l, g_base, k))

            # ---------------- the decoder ----------------
            # 5-slot weight ring: tA wk/cq, tB wv/co, tC wq/ck+1,
            # tD wo, tE cv (cross-v spans two LN fills)
            wk_t = wload(wk_d, 0, "tA")
            wv_t = wload(wv_d, 0, "tB")
            wq_t = wload(wq_d, 0, "tC")
            wo_t = wload(wo_d, 0, "tD")
            cv_t = wload(cv_d, 0, "tE")
            ck_t = None
            for l in range(nl):
                attn_projs(l, wq_t, wk_t, wv_t, VC_BQ, VC_BK, xBs, xBs)
                cq_t = wload(cq_d, l, "tA")
                co_t = wload(co_d, l, "tB")
                if l == 0:
                    ck_t = wload(ck_d, 0, "tC")
                attn_chains(l, wo_t, VC_CO, True, kTs, vs)
                if l + 1 < nl:
                    wo_t = wload(wo_d, l + 1, "tD")

                if l == 0:
                    def kcb0(o, ps):
                        nc.scalar.activation(
                            ckTs[o][:], ps[:], AF.Identity,
                            bias=vcol(0, VC_CBK, o))
                    f1 = (proj_fm_blocks(ck_t, memBs, KD, kcb0,
                                         alt_psum_obs=(0,))
                          + proj_v_blocks(cv_t, memBs, cvs))
                else:
                    # tail chunk of cross-v(l), head done under LN3
                    f1 = proj_v_blocks(cv_t, memBs, cvs, chunks=(1,))
                layernorm(l, VC_L1G, VC_L1B,
                          fill_pre=f1[:1] if l == 0 else (),
                          fill=f1[1:] if l == 0 else f1)
                if l + 1 < nl:
                    ck_t = wload(ck_d, l + 1, "tC")
                    cv_t = wload(cv_d, l + 1, "tE")

                def qcb(o, ps):
                    nc.vector.tensor_scalar_add(qTs[o][:], ps[:],
                                                vcol(l, VC_CBQ, o))
                proj_fm(cq_t, xBs, KD, qcb)
                if l + 1 < nl:
                    wk_t = wload(wk_d, l + 1, "tA")
                else:
                    wp_t = wload(wp_d, 0, "tA")
                w1_t = wload(w1_d, l, "w1", KD * DFF)
                attn_chains(l, co_t, VC_CCO, False, ckTs, cvs)
                if l + 1 < nl:
                    wv_t = wload(wv_d, l + 1, "tB")
                w2_t = wload(w2_d, l, "w2", KF * D)

                if l + 1 < nl:
                    def kcb2(o, ps, l=l):
                        nc.scalar.activation(
                            ckTs[o][:], ps[:], AF.Identity,
                            bias=vcol(l + 1, VC_CBK, o))
                    f2 = proj_fm_blocks(ck_t, memBs, KD, kcb2,
                                        alt_psum_obs=(0,))
                else:
                    f2 = []
                layernorm(l, VC_L2G, VC_L2B,
                          fill_pre=f2[:1], fill=f2[1:])
                if l + 1 < nl:
                    wq_t = wload(wq_d, l + 1, "tC")

                def ffcb(o, ps):
                    nc.scalar.activation(hTs[o][:], ps[:], AF.Relu,
                                         bias=vcol(l, VC_B1, o))
                proj_fm(w1_t, xBs, KF, ffcb, cw=DFF)

                def f2cb(o, ps):
                    nc.vector.scalar_tensor_tensor(
                        xBs[o][:], ps[:], vcol(l, VC_B2, o), xBs[o][:],
                        ALU.add, ALU.add)
                proj_fm(w2_t, hTs, KD, f2cb)

                # head chunk of cross-v(l+1); tail follows under LN1
                f3 = (proj_v_blocks(cv_t, memBs, cvs, chunks=(0,),
                                    alt_psum_t0s=(0,))
                      if l + 1 < nl else [])
                layernorm(l, VC_L3G, VC_L3B,
                          fill_pre=f3[:1], fill=f3[1:],
                          hi=(l + 1 == nl))

            # final projection (ring of 3 staging tiles, DMA per block)
            def outcb(o, ps):
                ot = tmp.tile([P, T], F32, tag="ot", bufs=3, name=f"ot{o}")
                nc.scalar.activation(ot[:], ps[:], AF.Identity,
                                     bias=bp_sb[:, o:o + 1])
                nc.sync.dma_start(out_d[o * P:(o + 1) * P, :], ot[:])
            proj_fm(wp_t, xBs, KD, outcb)

    nc.finalize()
    return nc


_CACHE = {}


def _get_nc(nl=L):
    if nl not in _CACHE:
        _CACHE[nl] = build_nc(nl)
    return _CACHE[nl]


def _sinusoidal_pe(seq, d):
    pos = np.arange(seq, dtype=np.float32)[:, None]
    div = np.exp(np.arange(0, d, 2, dtype=np.float32)
                 * (-np.log(10000.0) / d))
    pe = np.zeros((seq, d), np.float32)
    pe[:, 0::2] = np.sin(pos * div)
    pe[:, 1::2] = np.cos(pos * div)
    return pe


def _pack_cols(*vecs):
    """stack [768]/[2048] vectors as [128, k] column groups"""
    cols = []
    for v in vecs:
        cols.append(np.asarray(v, np.float32).reshape(-1, P).T)
    return np.concatenate(cols, axis=1)


def _repack(w):
    """[nl, d_out, d_in] -> transposed+tiled [nl, 128, k*d_out].

    Tile k holds input-dims [k*128,(k+1)*128) on partitions with all
    output columns adjacent: out[:, k*d_out + o] = w[o, k*128 + p]."""
    bf16 = mybir.dt.np(BF)
    wt = np.transpose(w, (0, 2, 1))          # [nl, d_in, d_out]
    nl_, din, dout = wt.shape
    k = din // P
    wt = wt.reshape(nl_, k, P, dout).transpose(0, 2, 1, 3)
    return np.ascontiguousarray(wt.reshape(nl_, P, k * dout).astype(bf16))


def prepare(inputs, nl=L):
    bf16 = mybir.dt.np(BF)
    f = lambda k: np.asarray(inputs[k], np.float32)
    enc = f("encoded_patches")
    pe = _sinusoidal_pe(SB, D)
    xpe = enc + pe[None]

    Wsi, bsi = f("W_self_in"), f("b_self_in")
    Wso, bso = f("W_self_out"), f("b_self_out")
    Wci, bci = f("W_cross_in"), f("b_cross_in")
    Wco, bco = f("W_cross_out"), f("b_cross_out")
    scale = 1.0 / np.sqrt(HD)

    shared = {}
    shared["wq"] = _repack(Wsi[:nl, :D] * scale)
    shared["wk"] = _repack(Wsi[:nl, D:2 * D])
    shared["wv"] = _repack(Wsi[:nl, 2 * D:])
    shared["wo"] = _repack(Wso[:nl])
    shared["cq"] = _repack(Wci[:nl, :D] * scale)
    shared["ck"] = _repack(Wci[:nl, D:2 * D])
    shared["cv"] = _repack(Wci[:nl, 2 * D:])
    shared["co"] = _repack(Wco[:nl])
    shared["w1"] = _repack(f("W_ff1")[:nl])
    shared["w2"] = _repack(f("W_ff2")[:nl])
    shared["wp"] = _repack(f("W_patch")[None])
    shared["bp"] = np.concatenate(
        [_pack_cols(f("b_patch")), np.full((P, 1), EPS, np.float32)], axis=1)
    shared["ones"] = np.concatenate(
        [np.ones((P, P), np.float32), np.zeros((P, P), np.float32),
         np.full((P, P), 1.0 / D, np.float32), np.eye(P, dtype=np.float32)],
        axis=1).astype(bf16)

    # attention-output bias folds: c_o = Wo @ bv + b_out
    vecs = []
    for l in range(nl):
        bv = bsi[l, 2 * D:]
        cbv = bci[l, 2 * D:]
        vecs.append(_pack_cols(
            bsi[l, :D] * scale, bsi[l, D:2 * D],
            bci[l, :D] * scale, bci[l, D:2 * D],
            Wso[l] @ bv + bso[l], Wco[l] @ cbv + bco[l],
            f("b_ff2")[l],
            f("ln1_g")[l], f("ln1_b")[l],
            f("ln2_g")[l], f("ln2_b")[l],
            f("ln3_g")[l], f("ln3_b")[l],
            f("b_ff1")[l]))
    shared["vec"] = np.ascontiguousarray(np.stack(vecs))

    # combined causal mask over the [256 | 128] score strip:
    # cols 0:256 -> k-block0 vs q in [0,256); cols 256:384 -> k-block1
    # vs q in [128,256)
    kp = np.arange(P)[:, None]
    q = np.arange(SB)[None, :]
    m0 = (kp <= q).astype(np.float32)
    q2 = np.arange(P)[None, :]
    m1 = (kp <= q2).astype(np.float32)
    strip = np.concatenate([m0, m1], axis=1)
    shared["maskT"] = np.ascontiguousarray(
        np.concatenate([strip, strip], axis=1).astype(bf16))

    in_maps = []
    for c in range(NCORES):
        b0 = c * NB
        m = dict(shared)
        xc = np.ascontiguousarray(xpe[b0:b0 + NB].reshape(T, D).T)
        m["xpeB"] = xc.astype(bf16)
        m["memB"] = np.ascontiguousarray(
            enc[b0:b0 + NB].reshape(T, D).T.astype(bf16))
        in_maps.append(m)
    return in_maps


def gather(results):
    outs = []
    for r in results:
        o = np.asarray(r["outT"])          # [768, 512]
        outs.append(o.T.reshape(NB, SB, D))
    full = np.concatenate(outs, axis=0)     # [16, 256, 768]
    out = full.reshape(-1, 256, 256, 3)
    return np.ascontiguousarray(np.transpose(out, (0, 3, 1, 2)))


def run(inputs, trace=False, nl=L):
    nc = _get_nc(nl)
    in_maps = prepare(inputs, nl)
    res = run_bass_kernel_spmd(nc, in_maps, list(range(NCORES)),
                               trace=trace)
    return gather(res.results), res


def kernel(**inputs):
    out, _ = run(inputs)
    return out



# revision 37
# speedup vs baseline: 1.0413x; 1.0413x over previous
"""Trainium2 Bass kernel for nn_Decoder_63505386438998.

6-layer post-norm transformer decoder (self-attn + cross-attn + FFN),
B=16, S=256, D=768, H=12, DFF=2048, fp32 in/out.

Strategy: pure data parallel — 8 cores x 2 batches each, weights
replicated, no collectives. Feature-major [feature, token] layout so
every linear is a chain of 128x128xN matmuls with weights stationary.

v2 over the baseline:
- Weights are host-repacked to [128, k*cols] so each matrix loads with
  ONE large contiguous DMA (1-3 MB) instead of dozens of 64 KB strided
  slices; weight tiles are SBUF-resident, slot-recycled by tag.
- Attention runs as per-(head-pair, batch) chains: one fused exp over
  the whole score strip, one combined causal-mask multiply, one shared
  reciprocal per pair, and the out-projection interleaved (lagged one
  pair) into 3 packed PSUM banks. This keeps the tensor engine dense
  through softmax so the HAM clock stays at 2.4 GHz.
- Cross-attn K/V projections are emitted as fill work under LN1's
  serial chain; weight prefetches are emitted as soon as the previous
  tenant of their SBUF slot is dead.
"""

import sys

import numpy as np

try:
    import concourse.bass as bass
except ImportError:  # toolchain lives here in the execution container
    sys.path.insert(0, "/opt/trn_rl_repo")
    import concourse.bass as bass

import concourse.bacc as bacc
import concourse.mybir as mybir
from concourse import tile
from concourse.bass_utils import run_bass_kernel_spmd

P = 128
NB = 2            # batches per core
SB = 256          # sequence length
T = NB * SB       # tokens per core = 512
D = 768
KD = D // P       # 6 feature tiles
DFF = 2048
KF = DFF // P     # 16 hidden tiles
H = 12
HD = 64
NP = H // 2       # 6 head pairs
L = 6
NCORES = 8
EPS = 1e-5

F32 = mybir.dt.float32
BF = mybir.dt.bfloat16
AF = mybir.ActivationFunctionType
ALU = mybir.AluOpType

# packed per-layer vector columns (layout [L, 128, NV]); value = col base
VC_BQ, VC_BK, VC_CBQ, VC_CBK, VC_CO, VC_CCO = 0, 6, 12, 18, 24, 30
VC_B2, VC_L1G, VC_L1B, VC_L2G, VC_L2B = 36, 42, 48, 54, 60
VC_L3G, VC_L3B, VC_B1 = 66, 72, 78
NV = 78 + KF  # 94


def build_nc(nl=L):
    nc = bacc.Bacc(None, target_bir_lowering=False)

    xpeB_d = nc.declare_dram_parameter("xpeB", [D, T], BF, False)
    memB_d = nc.declare_dram_parameter("memB", [D, T], BF, False)
    # weights repacked host-side: [nl, 128, KD*D] etc (k-tiles adjacent)
    wq_d = nc.declare_dram_parameter("wq", [nl, P, KD * D], BF, False)
    wk_d = nc.declare_dram_parameter("wk", [nl, P, KD * D], BF, False)
    wv_d = nc.declare_dram_parameter("wv", [nl, P, KD * D], BF, False)
    wo_d = nc.declare_dram_parameter("wo", [nl, P, KD * D], BF, False)
    cq_d = nc.declare_dram_parameter("cq", [nl, P, KD * D], BF, False)
    ck_d = nc.declare_dram_parameter("ck", [nl, P, KD * D], BF, False)
    cv_d = nc.declare_dram_parameter("cv", [nl, P, KD * D], BF, False)
    co_d = nc.declare_dram_parameter("co", [nl, P, KD * D], BF, False)
    w1_d = nc.declare_dram_parameter("w1", [nl, P, KD * DFF], BF, False)
    w2_d = nc.declare_dram_parameter("w2", [nl, P, KF * D], BF, False)
    wp_d = nc.declare_dram_parameter("wp", [1, P, KD * D], BF, False)
    vec_d = nc.declare_dram_parameter("vec", [nl, P, NV], F32, False)
    bp_d = nc.declare_dram_parameter("bp", [P, KD + 1], F32, False)
    ones_d = nc.declare_dram_parameter("ones", [P, 4 * P], BF, False)
    mask_d = nc.declare_dram_parameter("maskT", [P, 2 * (SB + P)], BF,
                                       False)
    out_d = nc.declare_dram_parameter("outT", [D, T], F32, True)

    with tile.TileContext(nc) as tc:
        with (
            tc.tile_pool(name="res", bufs=1) as res,
            tc.tile_pool(name="wpool", bufs=1) as wpool,
            tc.tile_pool(name="tmp", bufs=2) as tmp,
            tc.tile_pool(name="attn", bufs=6) as attn,
            tc.tile_pool(name="pp", bufs=3, space="PSUM") as pp,
            tc.tile_pool(name="patt", bufs=1, space="PSUM") as patt,
        ):
            # ---------------- resident tiles ----------------
            xBs = [res.tile([P, T], BF, tag=f"xB{i}", name=f"xB{i}")
                   for i in range(KD)]
            memBs = [res.tile([P, T], BF, tag=f"memB{i}", name=f"memB{i}")
                     for i in range(KD)]
            qTs = [res.tile([P, T], BF, tag=f"qT{i}", name=f"qT{i}")
                   for i in range(KD)]
            kTs = [res.tile([P, T], BF, tag=f"kT{i}", name=f"kT{i}")
                   for i in range(KD)]
            vs = [res.tile([P, D], BF, tag=f"v{i}", name=f"v{i}")
                  for i in range(NB * 2)]
            ckTs = [res.tile([P, T], BF, tag=f"ckT{i}", name=f"ckT{i}")
                    for i in range(KD)]
            cvs = [res.tile([P, D], BF, tag=f"cv{i}", name=f"cv{i}")
                   for i in range(NB * 2)]
            aTs = [res.tile([P, T], BF, tag=f"aT{i}", name=f"aT{i}")
                   for i in range(KD)]
            hTs = [res.tile([P, T], BF, tag=f"hT{i}", name=f"hT{i}")
                   for i in range(KF)]
            vec_sb = res.tile([P, nl * NV], F32, tag="vec", name="vec")
            maskc = res.tile([P, 2 * (SB + P)], BF, tag="maskc",
                             name="maskc")
            bp_sb = res.tile([P, KD + 1], F32, tag="bp", name="bp")
            dumt = res.tile([P, 1], F32, tag="dumt", name="dumt")
            ones3 = res.tile([P, 4 * P], BF, tag="ones", name="ones")
            ones = ones3[:, 0:P]
            zeros = ones3[:, P:2 * P]
            oD = ones3[:, 2 * P:3 * P]  # ones / D for LN stats
            ident = ones3[:, 3 * P:4 * P]

            nc.sync.dma_start(ones3[:], ones_d[:])
            eps_sb = bp_sb[:, KD:KD + 1]
            for i in range(KD):
                nc.sync.dma_start(xBs[i][:], xpeB_d[i * P:(i + 1) * P, :])
                nc.sync.dma_start(memBs[i][:], memB_d[i * P:(i + 1) * P, :])
            nc.sync.dma_start(maskc[:], mask_d[:])
            # warm the exp table set before the first attention
            nc.scalar.activation(dumt[:], ones3[:, 0:1], AF.Exp)
            for l in range(nl):
                nc.sync.dma_start(vec_sb[:, l * NV:(l + 1) * NV], vec_d[l])
            nc.sync.dma_start(bp_sb[:], bp_d[:])

            def vcol(l, base, i):
                return vec_sb[:, l * NV + base + i:l * NV + base + i + 1]

            # ---- weight staging: one contiguous DMA per matrix ----
            def wload(wdram, l, tag, width=KD * D):
                wt = wpool.tile([P, width], BF, tag=tag, name=tag)
                nc.sync.dma_start(wt[:], wdram[l])
                return wt

            # ------------- building blocks -------------
            def proj_fm_blocks(wt, src, nout, out_cb, cw=D,
                               alt_psum_obs=()):
                """thunk per o-block of 2 psum tiles; emitting a thunk
                lays down its matmuls + consume ops. Blocks listed in
                alt_psum_obs borrow the attention sm-slots in PSUM so
                they can run while the po banks are still draining."""
                nko = len(src)

                def blk(ob):
                    o0 = 2 * ob
                    nt = min(2, nout - o0)
                    if ob in alt_psum_obs:
                        pss = [patt.tile([P, T], F32, tag="sm", bufs=2,
                                         name=f"fp{o0 + j}")
                               for j in range(nt)]
                    else:
                        pss = [pp.tile([P, T], F32, tag="pp",
                                       name=f"pp{o0 + j}")
                               for j in range(nt)]
                    for k in range(nko):
                        for j in range(nt):
                            o = o0 + j
                            nc.tensor.matmul(
                                pss[j][:],
                                wt[:, k * cw + o * P:k * cw + (o + 1) * P],
                                src[k][:],
                                start=(k == 0),
                                stop=(k == nko - 1))
                    for j in range(nt):
                        out_cb(o0 + j, pss[j])

                return [lambda ob=ob: blk(ob)
                        for ob in range((nout + 1) // 2)]

            def proj_fm(wt, src, nout, out_cb, cw=D):
                """out[o,t] = sum_k w[:,k*cw+o*128].T @ src[k]"""
                for th in proj_fm_blocks(wt, src, nout, out_cb, cw):
                    th()

            def proj_v_blocks(wt, src, dst, chunks=(0, 1),
                              alt_psum_t0s=()):
                """dst[bt][t, o] (token-major [128, D] tiles) from
                feature-major src; weights are the moving operand.
                One thunk per (chunk, token-pair)."""
                def blk(c0, csz, t0):
                    if c0 == 0 and t0 in alt_psum_t0s:
                        pss = [patt.tile([P, csz], F32, tag="sm", bufs=2,
                                         name=f"fv{t0 + i}")
                               for i in range(2)]
                    else:
                        pss = [pp.tile([P, csz], F32, tag="pp",
                                       name=f"vps{t0 + i}")
                               for i in range(2)]
                    for k in range(KD):
                        for i in range(2):
                            t = t0 + i
                            nc.tensor.matmul(
                                pss[i][:],
                                src[k][:, t * P:(t + 1) * P],
                                wt[:, k * D + c0:k * D + c0 + csz],
                                start=(k == 0), stop=(k == KD - 1))
                    for i in range(2):
                        nc.scalar.copy(dst[t0 + i][:, c0:c0 + csz],
                                       pss[i][:])

                thunks = []
                for ci, c0 in enumerate(range(0, D, 512)):
                    if ci not in chunks:
                        continue
                    csz = min(512, D - c0)
                    for t0 in range(0, 4, 2):
                        thunks.append(
                            lambda c0=c0, csz=csz, t0=t0: blk(c0, csz, t0))
                return thunks

            def proj_v(wt, src, dst, chunks=(0, 1)):
                for th in proj_v_blocks(wt, src, dst, chunks):
                    th()

            def attn_projs(l, wq_t, wk_t, wv_t, bq_base, bk_base,
                           qsrc, kvsrc):
                def kcb(o, ps):
                    nc.vector.tensor_scalar_add(kTs[o][:], ps[:],
                                                vcol(l, bk_base, o))
                def qcb(o, ps):
                    nc.vector.tensor_scalar_add(qTs[o][:], ps[:],
                                                vcol(l, bq_base, o))
                proj_fm(wk_t, kvsrc, KD, kcb)
                proj_v(wv_t, kvsrc, vs)
                proj_fm(wq_t, qsrc, KD, qcb)

            def attn_chains(l, wo_t, co_base, causal, kt, vt):
                """softmax chains per (head-pair, batch) with the
                out-projection interleaved, lagged one pair behind.

                The first out-projection writer of each po region uses
                start=True (zeroing the bank in-place), the last uses
                stop=True; the residual + bias add is folded into the
                consume via scalar_tensor_tensor."""
                W = SB + P if causal else 2 * SB  # score strip width
                for b in range(NB):
                    bq = slice(b * SB, (b + 1) * SB)
                    po = [pp.tile([P, T], F32, tag="pp", name=f"po{m}")
                          for m in range(3)]
                    for m in range(3):
                        nc.tensor.matmul(po[m][:], zeros, memBs[0][:],
                                         start=True, stop=False)

                    def outproj(j):
                        for o in range(KD):
                            m, half = o // 2, o % 2
                            nc.tensor.matmul(
                                po[m][:, half * SB:(half + 1) * SB],
                                wo_t[:, j * D + o * P:j * D + (o + 1) * P],
                                aTs[j][:, bq],
                                start=False, stop=(j == NP - 1))

                    for j in range(NP):
                        at2 = attn.tile([P, 4 * SB], BF, tag="at",
                                        name="at2")
                        for hh in range(2):  # even/odd head of pair j
                            off = 64 * hh
                            hsl = slice(off, off + 64)
                            sc = patt.tile([P, W], F32, tag="sc", bufs=2,
                                           name=f"sc{hh}")
                            for s in range(2):
                                lo = P if (causal and s == 1) else 0
                                w_q = SB - lo
                                ks = kt[j][hsl, b * SB + s * P:
                                           b * SB + (s + 1) * P]
                                qs = qTs[j][hsl, b * SB + lo:(b + 1) * SB]
                                nc.tensor.matmul(sc[:, s * SB:s * SB + w_q],
                                                 ks, qs,
                                                 tile_position=(off, 0))
                            nc.scalar.activation(
                                at2[:, hh * W:(hh + 1) * W], sc[:], AF.Exp)
                        if causal:
                            nc.vector.tensor_tensor(
                                at2[:, 0:2 * W], at2[:, 0:2 * W],
                                maskc[:], ALU.mult)
                        sm = patt.tile([P, 2 * SB], F32, tag="sm", bufs=2,
                                       name="sm")
                        for hh in range(2):
                            c0 = hh * SB
                            for s in range(2):
                                lo = P if (causal and s == 1) else 0
                                nc.tensor.matmul(
                                    sm[:, c0 + lo:c0 + SB], ones[:],
                                    at2[:, hh * W + s * SB:
                                         hh * W + (s + 1) * SB - lo],
                                    start=(hh == 0 and s == 0),
                                    stop=(s == 1))
                        rr = attn.tile([P, 2 * SB], F32, tag="rr", bufs=3,
                                       name="rr")
                        nc.vector.reciprocal_approx_fast(rr[:], sm[:])
                        ao = patt.tile([P, 2 * SB], F32, tag="ao", bufs=1,
                                       name="ao")
                        for hh in range(2):
                            if hh == 0:
                                c0, c1, osl = j * P, j * P + 64, slice(0, 64)
                            else:
                                c0, c1, osl = j * P, (j + 1) * P, slice(0, P)
                            for s in range(2):
                                lo = P if (causal and s == 1) else 0
                                nc.tensor.matmul(
                                    ao[osl, hh * SB + lo:(hh + 1) * SB],
                                    vt[b * 2 + s][:, c0:c1],
                                    at2[:, hh * W + s * SB:
                                         hh * W + (s + 1) * SB - lo],
                                    start=(s == 0), stop=(s == 1))
                        nc.vector.tensor_tensor(
                            aTs[j][0:64, bq], ao[0:64, 0:SB],
                            rr[0:64, 0:SB], ALU.mult)
                        nc.vector.tensor_tensor(
                            aTs[j][64:P, bq], ao[64:P, SB:2 * SB],
                            rr[64:P, SB:2 * SB], ALU.mult)
                        if j > 0:
                            outproj(j - 1)
                    outproj(NP - 1)
                    # consume: residual + folded bias in one DVE op
                    for o in range(KD):
                        m, half = o // 2, o % 2
                        ps = po[m][:, half * SB:(half + 1) * SB]
                        nc.vector.scalar_tensor_tensor(
                            xBs[o][:, bq], ps, vcol(l, co_base, o),
                            xBs[o][:, bq], ALU.add, ALU.add)

            def layernorm(l, g_base, b_base, fill_pre=(), fill=(),
                          hi=False, preload_rsqrt=True, preload_exp=True):
                # table preload: pull the rsqrt set in while stats run
                # (otherwise the ~2.7us ACT_TABLE_LOAD sits on the chain)
                if preload_rsqrt:
                    nc.scalar.activation(dumt[:], eps_sb,
                                         AF.Abs_reciprocal_sqrt,
                                         bias=eps_sb)
                # PE cover for the preceding consume boundary
                for th in fill_pre:
                    th()
                # stats via ones/D stationary: mu and E[x^2] directly
                mu = pp.tile([P, T], F32, tag="pp", name="mu")
                sq_ps = pp.tile([P, T], F32, tag="pp", name="sq_ps")
                for k in range(KD):
                    sq = tmp.tile([P, T], BF, tag="sq", name="sq")
                    nc.vector.tensor_tensor(sq[:], xBs[k][:], xBs[k][:],
                                            ALU.mult)
                    nc.tensor.matmul(mu[:], oD[:], xBs[k][:],
                                     start=(k == 0), stop=(k == KD - 1))
                    nc.tensor.matmul(sq_ps[:], oD[:], sq[:],
                                     start=(k == 0), stop=(k == KD - 1))
                # mu copy on scalar (off the DVE chain); sd straight
                # from PSUM on scalar in parallel. mu/inv in bf16: the
                # per-token scale error cancels at the next LN, so only
                # the network's last LN needs f32 (hi=True).
                sdt = F32 if hi else BF
                mu_sb = tmp.tile([P, T], sdt, tag="mu_sb", bufs=1,
                                 name="mu_sb")
                nc.scalar.copy(mu_sb[:], mu[:])
                sd = tmp.tile([P, T], F32, tag="sd", bufs=1, name="sd")
                nc.vector.tensor_tensor(sd[:], mu_sb[:], mu_sb[:], ALU.mult)
                m2 = tmp.tile([P, T], F32, tag="m2", bufs=1, name="m2")
                nc.vector.tensor_tensor(m2[:], sq_ps[:], sd[:], ALU.subtract)
                for th in fill:
                    th()  # independent PE work under the serial chain
                inv = tmp.tile([P, T], sdt, tag="inv", bufs=1, name="inv")
                nc.scalar.activation(inv[:], m2[:], AF.Abs_reciprocal_sqrt,
                                     bias=eps_sb)
                if preload_exp:
                    nc.scalar.activation(dumt[:], eps_sb, AF.Exp)
                for k in range(KD):
                    sub = tmp.tile([P, T], BF, tag="sub", bufs=3,
                                   name="sub")
                    nc.vector.tensor_tensor(sub[:], xBs[k][:], mu_sb[:],
                                            ALU.subtract)
                    t = tmp.tile([P, T], BF, tag="t", bufs=3, name="t")
                    nc.vector.tensor_tensor(t[:], sub[:], inv[:], ALU.mult)
                    if k % 2 == 0:
                        nc.vector.tensor_scalar(
                            xBs[k][:], t[:], vcol(l, g_base, k),
                            vcol(l, b_base, k), ALU.mult, ALU.add)
                    else:
                        nc.scalar.activation(xBs[k][:], t[:], AF.Identity,
                                             bias=vcol(l, b_base, k),
                                             scale=vcol(l, g_base, k))

            # ---------------- the decoder ----------------
            # 5-slot weight ring: tA wk/cq, tB wv/co, tC wq/ck+1,
            # tD wo, tE cv (cross-v spans two LN fills)
            wk_t = wload(wk_d, 0, "tA")
            wv_t = wload(wv_d, 0, "tB")
            wq_t = wload(wq_d, 0, "tC")
            wo_t = wload(wo_d, 0, "tD")
            cv_t = wload(cv_d, 0, "tE")
            ck_t = None
            for l in range(nl):
                attn_projs(l, wq_t, wk_t, wv_t, VC_BQ, VC_BK, xBs, xBs)
                cq_t = wload(cq_d, l, "tA")
                co_t = wload(co_d, l, "tB")
                if l == 0:
                    ck_t = wload(ck_d, 0, "tC")
                attn_chains(l, wo_t, VC_CO, True, kTs, vs)
                if l + 1 < nl:
                    wo_t = wload(wo_d, l + 1, "tD")

                if l == 0:
                    def kcb0(o, ps):
                        nc.scalar.activation(
                            ckTs[o][:], ps[:], AF.Identity,
                            bias=vcol(0, VC_CBK, o))
                    f1 = (proj_fm_blocks(ck_t, memBs, KD, kcb0,
                                         alt_psum_obs=(0,))
                          + proj_v_blocks(cv_t, memBs, cvs))
                else:
                    # tail chunk of cross-v(l), head done under LN3
                    f1 = proj_v_blocks(cv_t, memBs, cvs, chunks=(1,))
                layernorm(l, VC_L1G, VC_L1B,
                          fill_pre=f1[:1] if l == 0 else (),
                          fill=f1[1:] if l == 0 else f1)
                if l + 1 < nl:
                    ck_t = wload(ck_d, l + 1, "tC")
                    cv_t = wload(cv_d, l + 1, "tE")

                def qcb(o, ps):
                    nc.vector.tensor_scalar_add(qTs[o][:], ps[:],
                                                vcol(l, VC_CBQ, o))
                proj_fm(cq_t, xBs, KD, qcb)
                if l + 1 < nl:
                    wk_t = wload(wk_d, l + 1, "tA")
                else:
                    wp_t = wload(wp_d, 0, "tA")
                w1_t = wload(w1_d, l, "w1", KD * DFF)
                attn_chains(l, co_t, VC_CCO, False, ckTs, cvs)
                if l + 1 < nl:
                    wv_t = wload(wv_d, l + 1, "tB")
                w2_t = wload(w2_d, l, "w2", KF * D)

                if l + 1 < nl:
                    def kcb2(o, ps, l=l):
                        nc.scalar.activation(
                            ckTs[o][:], ps[:], AF.Identity,
                            bias=vcol(l + 1, VC_CBK, o))
                    f2 = proj_fm_blocks(ck_t, memBs, KD, kcb2,
                                        alt_psum_obs=(0,))
                else:
                    f2 = []
                layernorm(l, VC_L2G, VC_L2B,
                          fill_pre=f2[:1], fill=f2[1:],
                          preload_exp=False)
                if l + 1 < nl:
                    wq_t = wload(wq_d, l + 1, "tC")

                def ffcb(o, ps):
                    nc.scalar.activation(hTs[o][:], ps[:], AF.Relu,
                                         bias=vcol(l, VC_B1, o))
                proj_fm(w1_t, xBs, KF, ffcb, cw=DFF)

                def f2cb(o, ps):
                    nc.vector.scalar_tensor_tensor(
                        xBs[o][:], ps[:], vcol(l, VC_B2, o), xBs[o][:],
                        ALU.add, ALU.add)
                proj_fm(w2_t, hTs, KD, f2cb)

                # head chunk of cross-v(l+1); tail follows under LN1
                f3 = (proj_v_blocks(cv_t, memBs, cvs, chunks=(0,),
                                    alt_psum_t0s=(0,))
                      if l + 1 < nl else [])
                layernorm(l, VC_L3G, VC_L3B,
                          fill_pre=f3[:1], fill=f3[1:],
                          hi=(l + 1 == nl), preload_rsqrt=False,
                          preload_exp=(l + 1 < nl))

            # final projection (ring of 3 staging tiles, DMA per block)
            def outcb(o, ps):
                ot = tmp.tile([P, T], F32, tag="ot", bufs=3, name=f"ot{o}")
                nc.scalar.activation(ot[:], ps[:], AF.Identity,
                                     bias=bp_sb[:, o:o + 1])
                nc.sync.dma_start(out_d[o * P:(o + 1) * P, :], ot[:])
            proj_fm(wp_t, xBs, KD, outcb)

    nc.finalize()
    return nc


_CACHE = {}


def _get_nc(nl=L):
    if nl not in _CACHE:
        _CACHE[nl] = build_nc(nl)
    return _CACHE[nl]


def _sinusoidal_pe(seq, d):
    pos = np.arange(seq, dtype=np.float32)[:, None]
    div = np.exp(np.arange(0, d, 2, dtype=np.float32)
                 * (-np.log(10000.0) / d))
    pe = np.zeros((seq, d), np.float32)
    pe[:, 0::2] = np.sin(pos * div)
    pe[:, 1::2] = np.cos(pos * div)
    return pe


def _pack_cols(*vecs):
    """stack [768]/[2048] vectors as [128, k] column groups"""
    cols = []
    for v in vecs:
        cols.append(np.asarray(v, np.float32).reshape(-1, P).T)
    return np.concatenate(cols, axis=1)


def _repack(w):
    """[nl, d_out, d_in] -> transposed+tiled [nl, 128, k*d_out].

    Tile k holds input-dims [k*128,(k+1)*128) on partitions with all
    output columns adjacent: out[:, k*d_out + o] = w[o, k*128 + p]."""
    bf16 = mybir.dt.np(BF)
    wt = np.transpose(w, (0, 2, 1))          # [nl, d_in, d_out]
    nl_, din, dout = wt.shape
    k = din // P
    wt = wt.reshape(nl_, k, P, dout).transpose(0, 2, 1, 3)
    return np.ascontiguousarray(wt.reshape(nl_, P, k * dout).astype(bf16))


def prepare(inputs, nl=L):
    bf16 = mybir.dt.np(BF)
    f = lambda k: np.asarray(inputs[k], np.float32)
    enc = f("encoded_patches")
    pe = _sinusoidal_pe(SB, D)
    xpe = enc + pe[None]

    Wsi, bsi = f("W_self_in"), f("b_self_in")
    Wso, bso = f("W_self_out"), f("b_self_out")
    Wci, bci = f("W_cross_in"), f("b_cross_in")
    Wco, bco = f("W_cross_out"), f("b_cross_out")
    scale = 1.0 / np.sqrt(HD)

    shared = {}
    shared["wq"] = _repack(Wsi[:nl, :D] * scale)
    shared["wk"] = _repack(Wsi[:nl, D:2 * D])
    shared["wv"] = _repack(Wsi[:nl, 2 * D:])
    shared["wo"] = _repack(Wso[:nl])
    shared["cq"] = _repack(Wci[:nl, :D] * scale)
    shared["ck"] = _repack(Wci[:nl, D:2 * D])
    shared["cv"] = _repack(Wci[:nl, 2 * D:])
    shared["co"] = _repack(Wco[:nl])
    shared["w1"] = _repack(f("W_ff1")[:nl])
    shared["w2"] = _repack(f("W_ff2")[:nl])
    shared["wp"] = _repack(f("W_patch")[None])
    shared["bp"] = np.concatenate(
        [_pack_cols(f("b_patch")), np.full((P, 1), EPS, np.float32)], axis=1)
    shared["ones"] = np.concatenate(
        [np.ones((P, P), np.float32), np.zeros((P, P), np.float32),
         np.full((P, P), 1.0 / D, np.float32), np.eye(P, dtype=np.float32)],
        axis=1).astype(bf16)

    # attention-output bias folds: c_o = Wo @ bv + b_out
    vecs = []
    for l in range(nl):
        bv = bsi[l, 2 * D:]
        cbv = bci[l, 2 * D:]
        vecs.append(_pack_cols(
            bsi[l, :D] * scale, bsi[l, D:2 * D],
            bci[l, :D] * scale, bci[l, D:2 * D],
            Wso[l] @ bv + bso[l], Wco[l] @ cbv + bco[l],
            f("b_ff2")[l],
            f("ln1_g")[l], f("ln1_b")[l],
            f("ln2_g")[l], f("ln2_b")[l],
            f("ln3_g")[l], f("ln3_b")[l],
            f("b_ff1")[l]))
    shared["vec"] = np.ascontiguousarray(np.stack(vecs))

    # combined causal mask over the [256 | 128] score strip:
    # cols 0:256 -> k-block0 vs q in [0,256); cols 256:384 -> k-block1
    # vs q in [128,256)
    kp = np.arange(P)[:, None]
    q = np.arange(SB)[None, :]
    m0 = (kp <= q).astype(np.float32)
    q2 = np.arange(P)[None, :]
    m1 = (kp <= q2).astype(np.float32)
    strip = np.concatenate([m0, m1], axis=1)
    shared["maskT"] = np.ascontiguousarray(
        np.concatenate([strip, strip], axis=1).astype(bf16))

    in_maps = []
    for c in range(NCORES):
        b0 = c * NB
        m = dict(shared)
        xc = np.ascontiguousarray(xpe[b0:b0 + NB].reshape(T, D).T)
        m["xpeB"] = xc.astype(bf16)
        m["memB"] = np.ascontiguousarray(
            enc[b0:b0 + NB].reshape(T, D).T.astype(bf16))
        in_maps.append(m)
    return in_maps


def gather(results):
    outs = []
    for r in results:
        o = np.asarray(r["outT"])          # [768, 512]
        outs.append(o.T.reshape(NB, SB, D))
    full = np.concatenate(outs, axis=0)     # [16, 256, 768]
    out = full.reshape(-1, 256, 256, 3)
    return np.ascontiguousarray(np.transpose(out, (0, 3, 1, 2)))


def run(inputs, trace=False, nl=L):
    nc = _get_nc(nl)
    in_maps = prepare(inputs, nl)
    res = run_bass_kernel_spmd(nc, in_maps, list(range(NCORES)),
                               trace=trace)
    return gather(res.results), res


def kernel(**inputs):
    out, _ = run(inputs)
    return out



# revision 67
# speedup vs baseline: 1.0721x; 1.0296x over previous
"""Trainium2 Bass kernel for nn_Decoder_63505386438998.

6-layer post-norm transformer decoder (self-attn + cross-attn + FFN),
B=16, S=256, D=768, H=12, DFF=2048, fp32 in/out.

Strategy: pure data parallel — 8 cores x 2 batches each, weights
replicated, no collectives. Feature-major [feature, token] layout so
every linear is a chain of 128x128xN matmuls with weights stationary.

v2 over the baseline:
- Weights are host-repacked to [128, k*cols] so each matrix loads with
  ONE large contiguous DMA (1-3 MB) instead of dozens of 64 KB strided
  slices; weight tiles are SBUF-resident, slot-recycled by tag.
- Attention runs as per-(head-pair, batch) chains: one fused exp over
  the whole score strip, one combined causal-mask multiply, one shared
  reciprocal per pair, and the out-projection interleaved (lagged one
  pair) into 3 packed PSUM banks. This keeps the tensor engine dense
  through softmax so the HAM clock stays at 2.4 GHz.
- Cross-attn K/V projections are emitted as fill work under LN1's
  serial chain; weight prefetches are emitted as soon as the previous
  tenant of their SBUF slot is dead.
"""

import sys

import numpy as np

try:
    import concourse.bass as bass
except ImportError:  # toolchain lives here in the execution container
    sys.path.insert(0, "/opt/trn_rl_repo")
    import concourse.bass as bass

import concourse.bacc as bacc
import concourse.mybir as mybir
from concourse import tile
from concourse.bass_utils import run_bass_kernel_spmd

P = 128
NB = 2            # batches per core
SB = 256          # sequence length
T = NB * SB       # tokens per core = 512
D = 768
KD = D // P       # 6 feature tiles
DFF = 2048
KF = DFF // P     # 16 hidden tiles
H = 12
HD = 64
NP = H // 2       # 6 head pairs
L = 6
NCORES = 8
EPS = 1e-5

F32 = mybir.dt.float32
BF = mybir.dt.bfloat16
AF = mybir.ActivationFunctionType
ALU = mybir.AluOpType

# packed per-layer vector columns (layout [L, 128, NV]); value = col base
VC_BQ, VC_BK, VC_CBQ, VC_CBK, VC_CO, VC_CCO = 0, 6, 12, 18, 24, 30
VC_B2, VC_L1G, VC_L1B, VC_L2G, VC_L2B = 36, 42, 48, 54, 60
VC_L3G, VC_L3B, VC_B1 = 66, 72, 78
NV = 78 + KF  # 94


def build_nc(nl=L):
    nc = bacc.Bacc(None, target_bir_lowering=False)

    xpeB_d = nc.declare_dram_parameter("xpeB", [D, T], BF, False)
    memB_d = nc.declare_dram_parameter("memB", [D, T], BF, False)
    # weights repacked host-side: [nl, 128, KD*D] etc (k-tiles adjacent)
    wq_d = nc.declare_dram_parameter("wq", [nl, P, KD * D], BF, False)
    wk_d = nc.declare_dram_parameter("wk", [nl, P, KD * D], BF, False)
    wv_d = nc.declare_dram_parameter("wv", [nl, P, KD * D], BF, False)
    wo_d = nc.declare_dram_parameter("wo", [nl, P, KD * D], BF, False)
    cq_d = nc.declare_dram_parameter("cq", [nl, P, KD * D], BF, False)
    ck_d = nc.declare_dram_parameter("ck", [nl, P, KD * D], BF, False)
    cv_d = nc.declare_dram_parameter("cv", [nl, P, KD * D], BF, False)
    co_d = nc.declare_dram_parameter("co", [nl, P, KD * D], BF, False)
    w1_d = nc.declare_dram_parameter("w1", [nl, P, KD * DFF], BF, False)
    w2_d = nc.declare_dram_parameter("w2", [nl, P, KF * D], BF, False)
    wp_d = nc.declare_dram_parameter("wp", [1, P, KD * D], BF, False)
    vec_d = nc.declare_dram_parameter("vec", [nl, P, NV], F32, False)
    bp_d = nc.declare_dram_parameter("bp", [P, KD + 1], F32, False)
    ones_d = nc.declare_dram_parameter("ones", [P, 4 * P], BF, False)
    mask_d = nc.declare_dram_parameter("maskT", [P, 2 * (SB + P)], BF,
                                       False)
    out_d = nc.declare_dram_parameter("outT", [D, T], F32, True)

    with tile.TileContext(nc) as tc:
        with (
            tc.tile_pool(name="res", bufs=1) as res,
            tc.tile_pool(name="wpool", bufs=1) as wpool,
            tc.tile_pool(name="tmp", bufs=2) as tmp,
            tc.tile_pool(name="attn", bufs=6) as attn,
            tc.tile_pool(name="pp", bufs=3, space="PSUM") as pp,
            tc.tile_pool(name="patt", bufs=1, space="PSUM") as patt,
        ):
            # ---------------- resident tiles ----------------
            xBs = [res.tile([P, T], BF, tag=f"xB{i}", name=f"xB{i}")
                   for i in range(KD)]
            memBs = [res.tile([P, T], BF, tag=f"memB{i}", name=f"memB{i}")
                     for i in range(KD)]
            qTs = [res.tile([P, T], BF, tag=f"qT{i}", name=f"qT{i}")
                   for i in range(KD)]
            kTs = [res.tile([P, T], BF, tag=f"kT{i}", name=f"kT{i}")
                   for i in range(KD)]
            vs = [res.tile([P, D], BF, tag=f"v{i}", name=f"v{i}")
                  for i in range(NB * 2)]
            # cross-K/V double-buffered by layer parity so the next
            # layer's ck/cv projections can fill any LN serial phase
            ckTs_s = [[res.tile([P, T], BF, tag=f"ckT{s}_{i}",
                                name=f"ckT{s}_{i}") for i in range(KD)]
                      for s in range(2)]
            cvs_s = [[res.tile([P, D], BF, tag=f"cv{s}_{i}",
                               name=f"cv{s}_{i}") for i in range(NB * 2)]
                     for s in range(2)]
            aTs = [res.tile([P, T], BF, tag=f"aT{i}", name=f"aT{i}")
                   for i in range(KD)]
            hTs = [res.tile([P, T], BF, tag=f"hT{i}", name=f"hT{i}")
                   for i in range(KF)]
            vec_sb = res.tile([P, nl * NV], F32, tag="vec", name="vec")
            maskc = res.tile([P, 2 * (SB + P)], BF, tag="maskc",
                             name="maskc")
            bp_sb = res.tile([P, KD + 1], F32, tag="bp", name="bp")
            dumt = res.tile([P, 1], F32, tag="dumt", name="dumt")
            ones3 = res.tile([P, 4 * P], BF, tag="ones", name="ones")
            ones = ones3[:, 0:P]
            zeros = ones3[:, P:2 * P]
            oD = ones3[:, 2 * P:3 * P]  # ones / D for LN stats
            ident = ones3[:, 3 * P:4 * P]

            nc.sync.dma_start(ones3[:], ones_d[:])
            eps_sb = bp_sb[:, KD:KD + 1]
            for i in range(KD):
                nc.sync.dma_start(xBs[i][:], xpeB_d[i * P:(i + 1) * P, :])
                nc.sync.dma_start(memBs[i][:], memB_d[i * P:(i + 1) * P, :])
            nc.sync.dma_start(maskc[:], mask_d[:])
            # warm the exp table set before the first attention
            nc.scalar.activation(dumt[:], ones3[:, 0:1], AF.Exp)
            for l in range(nl):
                nc.sync.dma_start(vec_sb[:, l * NV:(l + 1) * NV], vec_d[l])
            nc.sync.dma_start(bp_sb[:], bp_d[:])

            def vcol(l, base, i):
                return vec_sb[:, l * NV + base + i:l * NV + base + i + 1]

            # ---- weight staging: one contiguous DMA per matrix ----
            def wload(wdram, l, tag, width=KD * D):
                wt = wpool.tile([P, width], BF, tag=tag, name=tag)
                nc.sync.dma_start(wt[:], wdram[l])
                return wt

            # ------------- building blocks -------------
            def proj_fm_blocks(wt, src, nout, out_cb, cw=D,
                               alt_psum_obs=(), per=2):
                """thunk per o-block of `per` psum tiles; emitting a
                thunk lays down its matmuls + consume ops. Blocks in
                alt_psum_obs borrow the attention sm-slots in PSUM so
                they can run while the po banks are still draining."""
                nko = len(src)

                def blk(ob):
                    o0 = per * ob
                    nt = min(per, nout - o0)
                    if ob in alt_psum_obs:
                        pss = [patt.tile([P, T], F32, tag="sm", bufs=2,
                                         name=f"fp{o0 + j}")
                               for j in range(nt)]
                    else:
                        pss = [pp.tile([P, T], F32, tag="pp",
                                       name=f"pp{o0 + j}")
                               for j in range(nt)]
                    for k in range(nko):
                        for j in range(nt):
                            o = o0 + j
                            nc.tensor.matmul(
                                pss[j][:],
                                wt[:, k * cw + o * P:k * cw + (o + 1) * P],
                                src[k][:],
                                start=(k == 0),
                                stop=(k == nko - 1))
                    for j in range(nt):
                        out_cb(o0 + j, pss[j])

                return [lambda ob=ob: blk(ob)
                        for ob in range((nout + per - 1) // per)]

            def proj_fm(wt, src, nout, out_cb, cw=D):
                """out[o,t] = sum_k w[:,k*cw+o*128].T @ src[k]"""
                for th in proj_fm_blocks(wt, src, nout, out_cb, cw):
                    th()

            def proj_v_blocks(wt, src, dst, chunks=(0, 1),
                              alt_psum_t0s=()):
                """dst[bt][t, o] (token-major [128, D] tiles) from
                feature-major src; weights are the moving operand.
                One thunk per (chunk, token-pair)."""
                def blk(c0, csz, t0):
                    if (c0, t0) in alt_psum_t0s:
                        pss = [patt.tile([P, 512], F32, tag="sm", bufs=2,
                                         name=f"fv{t0 + i}")[:, 0:csz]
                               for i in range(2)]
                    else:
                        pss = [pp.tile([P, csz], F32, tag="pp",
                                       name=f"vps{t0 + i}")[:]
                               for i in range(2)]
                    for k in range(KD):
                        for i in range(2):
                            t = t0 + i
                            nc.tensor.matmul(
                                pss[i][:],
                                src[k][:, t * P:(t + 1) * P],
                                wt[:, k * D + c0:k * D + c0 + csz],
                                start=(k == 0), stop=(k == KD - 1))
                    for i in range(2):
                        nc.scalar.copy(dst[t0 + i][:, c0:c0 + csz],
                                       pss[i][:])

                thunks = []
                for ci, c0 in enumerate(range(0, D, 512)):
                    if ci not in chunks:
                        continue
                    csz = min(512, D - c0)
                    for t0 in range(0, 4, 2):
                        thunks.append(
                            lambda c0=c0, csz=csz, t0=t0: blk(c0, csz, t0))
                return thunks

            def proj_v(wt, src, dst, chunks=(0, 1)):
                for th in proj_v_blocks(wt, src, dst, chunks):
                    th()

            def attn_projs(l, wq_t, wk_t, wv_t, bq_base, bk_base,
                           qsrc, kvsrc):
                def kcb(o, ps):
                    nc.vector.tensor_scalar_add(kTs[o][:], ps[:],
                                                vcol(l, bk_base, o))
                def qcb(o, ps):
                    nc.vector.tensor_scalar_add(qTs[o][:], ps[:],
                                                vcol(l, bq_base, o))
                proj_fm(wk_t, kvsrc, KD, kcb)
                proj_v(wv_t, kvsrc, vs)
                proj_fm(wq_t, qsrc, KD, qcb)

            def attn_chains(l, wo_t, co_base, causal, kt, vt):
                """softmax chains per (head-pair, batch) with the
                out-projection interleaved, lagged one pair behind.

                The first out-projection writer of each po region uses
                start=True (zeroing the bank in-place), the last uses
                stop=True; the residual + bias add is folded into the
                consume via scalar_tensor_tensor."""
                W = SB + P if causal else 2 * SB  # score strip width
                for b in range(NB):
                    bq = slice(b * SB, (b + 1) * SB)
                    po = [pp.tile([P, T], F32, tag="pp", name=f"po{m}")
                          for m in range(3)]
                    for m in range(3):
                        nc.tensor.matmul(po[m][:], zeros, memBs[0][:],
                                         start=True, stop=False)

                    def outproj(j):
                        for o in range(KD):
                            m, half = o // 2, o % 2
                            nc.tensor.matmul(
                                po[m][:, half * SB:(half + 1) * SB],
                                wo_t[:, j * D + o * P:j * D + (o + 1) * P],
                                aTs[j][:, bq],
                                start=False, stop=(j == NP - 1))

                    def sc_exp(j):
                        """scores + exp for pair j"""
                        at2 = attn.tile([P, 4 * SB], BF, tag="at",
                                        bufs=4, name="at2")
                        for hh in range(2):  # even/odd head of pair j
                            off = 64 * hh
                            hsl = slice(off, off + 64)
                            sc = patt.tile([P, W], F32, tag="sc", bufs=2,
                                           name=f"sc{hh}")
                            for s in range(2):
                                lo = P if (causal and s == 1) else 0
                                w_q = SB - lo
                                ks = kt[j][hsl, b * SB + s * P:
                                           b * SB + (s + 1) * P]
                                qs = qTs[j][hsl, b * SB + lo:(b + 1) * SB]
                                nc.tensor.matmul(sc[:, s * SB:s * SB + w_q],
                                                 ks, qs,
                                                 tile_position=(off, 0))
                            nc.scalar.activation(
                                at2[:, hh * W:(hh + 1) * W], sc[:], AF.Exp)
                        return at2

                    def mask(at2):
                        if causal:
                            nc.vector.tensor_tensor(
                                at2[:, 0:2 * W], at2[:, 0:2 * W],
                                maskc[:], ALU.mult)

                    for j in range(NP):
                        at2 = sc_exp(j)
                        mask(at2)
                        sm = patt.tile([P, 2 * SB], F32, tag="sm", bufs=2,
                                       name="sm")
                        for hh in range(2):
                            c0 = hh * SB
                            for s in range(2):
                                lo = P if (causal and s == 1) else 0
                                nc.tensor.matmul(
                                    sm[:, c0 + lo:c0 + SB], ones[:],
                                    at2[:, hh * W + s * SB:
                                         hh * W + (s + 1) * SB - lo],
                                    start=(hh == 0 and s == 0),
                                    stop=(s == 1))
                        rr = attn.tile([P, 2 * SB], F32, tag="rr", bufs=2,
                                       name="rr")
                        nc.vector.reciprocal_approx_fast(rr[:], sm[:])
                        ao = patt.tile([P, 2 * SB], F32, tag="ao", bufs=1,
                                       name="ao")
                        for hh in range(2):
                            if hh == 0:
                                c0, c1, osl = j * P, j * P + 64, slice(0, 64)
                            else:
                                c0, c1, osl = j * P, (j + 1) * P, slice(0, P)
                            for s in range(2):
                                lo = P if (causal and s == 1) else 0
                                nc.tensor.matmul(
                                    ao[osl, hh * SB + lo:(hh + 1) * SB],
                                    vt[b * 2 + s][:, c0:c1],
                                    at2[:, hh * W + s * SB:
                                         hh * W + (s + 1) * SB - lo],
                                    start=(s == 0), stop=(s == 1))
                        nc.vector.tensor_tensor(
                            aTs[j][0:64, bq], ao[0:64, 0:SB],
                            rr[0:64, 0:SB], ALU.mult)
                        nc.vector.tensor_tensor(
                            aTs[j][64:P, bq], ao[64:P, SB:2 * SB],
                            rr[64:P, SB:2 * SB], ALU.mult)
                        if j > 0:
                            outproj(j - 1)
                    outproj(NP - 1)
                    # consume: residual + folded bias in one DVE op
                    for o in range(KD):
                        m, half = o // 2, o % 2
                        ps = po[m][:, half * SB:(half + 1) * SB]
                        nc.vector.scalar_tensor_tensor(
                            xBs[o][:, bq], ps, vcol(l, co_base, o),
                            xBs[o][:, bq], ALU.add, ALU.add)

            def layernorm(l, g_base, b_base, fill_pre=(), fill=(),
                          fill_late=(), hi=False, preload_rsqrt=True,
                          preload_exp=True):
                # table preload: pull the rsqrt set in while stats run
                # (otherwise the ~2.7us ACT_TABLE_LOAD sits on the chain)
                if preload_rsqrt:
                    nc.scalar.activation(dumt[:], eps_sb,
                                         AF.Abs_reciprocal_sqrt,
                                         bias=eps_sb)
                # PE cover for the preceding consume boundary
                for th in fill_pre:
                    th()
                # stats via ones/D stationary: mu and E[x^2] directly.
                # mu group first so the mu copy/sd hide under the sq
                # matmuls.
                mu = pp.tile([P, T], F32, tag="pp", name="mu")
                sq_ps = pp.tile([P, T], F32, tag="pp", name="sq_ps")
                sqs = []
                for k in range(KD):
                    sq = tmp.tile([P, T], BF, tag="sq", bufs=4,
                                  name="sq")
                    nc.vector.tensor_tensor(sq[:], xBs[k][:], xBs[k][:],
                                            ALU.mult)
                    sqs.append(sq)
                    nc.tensor.matmul(mu[:], oD[:], xBs[k][:],
                                     start=(k == 0), stop=(k == KD - 1))
                for k in range(KD):
                    nc.tensor.matmul(sq_ps[:], oD[:], sqs[k][:],
                                     start=(k == 0), stop=(k == KD - 1))
                # mu copy on scalar (off the DVE chain); sd straight
                # from PSUM on scalar in parallel. mu/inv in bf16: the
                # per-token scale error cancels at the next LN, so only
                # the network's last LN needs f32 (hi=True).
                sdt = F32 if hi else BF
                mu_sb = tmp.tile([P, T], sdt, tag="mu_sb", bufs=1,
                                 name="mu_sb")
                nc.scalar.copy(mu_sb[:], mu[:])
                sd = tmp.tile([P, T], F32, tag="sd", bufs=1, name="sd")
                nc.vector.tensor_tensor(sd[:], mu_sb[:], mu_sb[:], ALU.mult)
                m2 = tmp.tile([P, T], F32, tag="m2", bufs=1, name="m2")
                nc.vector.tensor_tensor(m2[:], sq_ps[:], sd[:], ALU.subtract)
                for th in fill:
                    th()  # independent PE work under the serial chain
                inv = tmp.tile([P, T], sdt, tag="inv", bufs=1, name="inv")
                nc.scalar.activation(inv[:], m2[:], AF.Abs_reciprocal_sqrt,
                                     bias=eps_sb)
                if preload_exp:
                    nc.scalar.activation(dumt[:], eps_sb, AF.Exp)
                for k in range(KD):
                    sub = tmp.tile([P, T], BF, tag="sub", bufs=3,
                                   name="sub")
                    nc.vector.tensor_tensor(sub[:], xBs[k][:], mu_sb[:],
                                            ALU.subtract)
                    t = tmp.tile([P, T], BF, tag="t", bufs=3, name="t")
                    nc.vector.tensor_tensor(t[:], sub[:], inv[:], ALU.mult)
                    if k % 2 == 0:
                        nc.vector.tensor_scalar(
                            xBs[k][:], t[:], vcol(l, g_base, k),
                            vcol(l, b_base, k), ALU.mult, ALU.add)
                    else:
                        nc.scalar.activation(xBs[k][:], t[:], AF.Identity,
                                             bias=vcol(l, b_base, k),
                                             scale=vcol(l, g_base, k))
                for th in fill_late:
                    th()  # PE cover for the apply-phase trickle

            # ---------------- the decoder ----------------
            # 5-slot weight ring: tA wk/cq, tB wv/co, tC wq/ck+1,
            # tD wo, tE cv (cross-v spans two LN fills)
            wk_t = wload(wk_d, 0, "tA")
            wv_t = wload(wv_d, 0, "tB")
            wq_t = wload(wq_d, 0, "tC")
            wo_t = wload(wo_d, 0, "tD")
            cv_t = wload(cv_d, 0, "tE")
            ck_t = None
            for l in range(nl):
                attn_projs(l, wq_t, wk_t, wv_t, VC_BQ, VC_BK, xBs, xBs)
                cq_t = wload(cq_d, l, "tA")
                co_t = wload(co_d, l, "tB")
                if l == 0:
                    ck_t = wload(ck_d, 0, "tC")
                attn_chains(l, wo_t, VC_CO, True, kTs, vs)
                if l + 1 < nl:
                    wo_t = wload(wo_d, l + 1, "tD")

                def kcbn(o, ps, tl=(l if l == 0 else l + 1)):
                    nc.scalar.activation(
                        ckTs_s[tl % 2][o][:], ps[:], AF.Identity,
                        bias=vcol(tl, VC_CBK, o))
                if l == 0:
                    f1 = (proj_fm_blocks(ck_t, memBs, KD, kcbn,
                                         alt_psum_obs=(0,))
                          + proj_v_blocks(cv_t, memBs, cvs_s[0],
                                          alt_psum_t0s=((512, 2),)))
                elif l + 1 < nl:
                    # next layer's cross-K under this LN's serial chain
                    ck_t = wload(ck_d, l + 1, "tC")
                    f1 = proj_fm_blocks(ck_t, memBs, KD, kcbn,
                                        alt_psum_obs=(0,))
                else:
                    f1 = []
                layernorm(l, VC_L1G, VC_L1B,
                          fill_pre=f1[:1], fill=f1[1:])
                if l == 0 and l + 1 < nl:
                    ck_t = wload(ck_d, 1, "tC")
                if l + 1 < nl:
                    cv_t = wload(cv_d, l + 1, "tE")

                def qcb(o, ps):
                    nc.vector.tensor_scalar_add(qTs[o][:], ps[:],
                                                vcol(l, VC_CBQ, o))
                proj_fm(cq_t, xBs, KD, qcb)
                if l + 1 < nl:
                    wk_t = wload(wk_d, l + 1, "tA")
                else:
                    wp_t = wload(wp_d, 0, "tA")
                w1_t = wload(w1_d, l, "w1", KD * DFF)
                attn_chains(l, co_t, VC_CCO, False,
                            ckTs_s[l % 2], cvs_s[l % 2])
                if l + 1 < nl:
                    wv_t = wload(wv_d, l + 1, "tB")
                w2_t = wload(w2_d, l, "w2", KF * D)

                if l + 1 < nl:
                    if l == 0:
                        def kcb2(o, ps):
                            nc.scalar.activation(
                                ckTs_s[1][o][:], ps[:], AF.Identity,
                                bias=vcol(1, VC_CBK, o))
                        f2 = proj_fm_blocks(ck_t, memBs, KD, kcb2,
                                            alt_psum_obs=(0,))
                    else:
                        # next layer's cross-V head chunk
                        f2 = proj_v_blocks(cv_t, memBs,
                                           cvs_s[(l + 1) % 2],
                                           chunks=(0,),
                                           alt_psum_t0s=((0, 0),))
                else:
                    f2 = []
                layernorm(l, VC_L2G, VC_L2B,
                          fill_pre=f2[:1], fill=f2[1:],
                          preload_exp=False)
                if l + 1 < nl:
                    wq_t = wload(wq_d, l + 1, "tC")

                def ffcb(o, ps):
                    nc.scalar.activation(hTs[o][:], ps[:], AF.Relu,
                                         bias=vcol(l, VC_B1, o))
                proj_fm(w1_t, xBs, KF, ffcb, cw=DFF)

                def f2cb(o, ps):
                    nc.vector.scalar_tensor_tensor(
                        xBs[o][:], ps[:], vcol(l, VC_B2, o), xBs[o][:],
                        ALU.add, ALU.add)
                proj_fm(w2_t, hTs, KD, f2cb)

                # next layer's cross-V tail chunk (l=0: full cross-V)
                if l + 1 < nl:
                    f3 = proj_v_blocks(
                        cv_t, memBs, cvs_s[(l + 1) % 2],
                        chunks=(0, 1) if l == 0 else (1,),
                        alt_psum_t0s=((0, 0), (512, 0)))
                else:
                    f3 = []
                layernorm(l, VC_L3G, VC_L3B,
                          fill_pre=f3[:1], fill=f3[1:],
                          hi=(l + 1 == nl), preload_rsqrt=False,
                          preload_exp=(l + 1 < nl))

            # final projection (ring of 3 staging tiles, DMA per block)
            def outcb(o, ps):
                ot = tmp.tile([P, T], F32, tag="ot", bufs=2, name=f"ot{o}")
                nc.scalar.activation(ot[:], ps[:], AF.Identity,
                                     bias=bp_sb[:, o:o + 1])
                nc.sync.dma_start(out_d[o * P:(o + 1) * P, :], ot[:])
            proj_fm(wp_t, xBs, KD, outcb)

    nc.finalize()
    return nc


_CACHE = {}


def _get_nc(nl=L):
    if nl not in _CACHE:
        _CACHE[nl] = build_nc(nl)
    return _CACHE[nl]


def _sinusoidal_pe(seq, d):
    pos = np.arange(seq, dtype=np.float32)[:, None]
    div = np.exp(np.arange(0, d, 2, dtype=np.float32)
                 * (-np.log(10000.0) / d))
    pe = np.zeros((seq, d), np.float32)
    pe[:, 0::2] = np.sin(pos * div)
    pe[:, 1::2] = np.cos(pos * div)
    return pe


def _pack_cols(*vecs):
    """stack [768]/[2048] vectors as [128, k] column groups"""
    cols = []
    for v in vecs:
        cols.append(np.asarray(v, np.float32).reshape(-1, P).T)
    return np.concatenate(cols, axis=1)


def _repack(w):
    """[nl, d_out, d_in] -> transposed+tiled [nl, 128, k*d_out].

    Tile k holds input-dims [k*128,(k+1)*128) on partitions with all
    output columns adjacent: out[:, k*d_out + o] = w[o, k*128 + p]."""
    bf16 = mybir.dt.np(BF)
    wt = np.transpose(w, (0, 2, 1))          # [nl, d_in, d_out]
    nl_, din, dout = wt.shape
    k = din // P
    wt = wt.reshape(nl_, k, P, dout).transpose(0, 2, 1, 3)
    return np.ascontiguousarray(wt.reshape(nl_, P, k * dout).astype(bf16))


def prepare(inputs, nl=L):
    bf16 = mybir.dt.np(BF)
    f = lambda k: np.asarray(inputs[k], np.float32)
    enc = f("encoded_patches")
    pe = _sinusoidal_pe(SB, D)
    xpe = enc + pe[None]

    Wsi, bsi = f("W_self_in"), f("b_self_in")
    Wso, bso = f("W_self_out"), f("b_self_out")
    Wci, bci = f("W_cross_in"), f("b_cross_in")
    Wco, bco = f("W_cross_out"), f("b_cross_out")
    scale = 1.0 / np.sqrt(HD)

    shared = {}
    shared["wq"] = _repack(Wsi[:nl, :D] * scale)
    shared["wk"] = _repack(Wsi[:nl, D:2 * D])
    shared["wv"] = _repack(Wsi[:nl, 2 * D:])
    shared["wo"] = _repack(Wso[:nl])
    shared["cq"] = _repack(Wci[:nl, :D] * scale)
    shared["ck"] = _repack(Wci[:nl, D:2 * D])
    shared["cv"] = _repack(Wci[:nl, 2 * D:])
    shared["co"] = _repack(Wco[:nl])
    shared["w1"] = _repack(f("W_ff1")[:nl])
    shared["w2"] = _repack(f("W_ff2")[:nl])
    shared["wp"] = _repack(f("W_patch")[None])
    shared["bp"] = np.concatenate(
        [_pack_cols(f("b_patch")), np.full((P, 1), EPS, np.float32)], axis=1)
    shared["ones"] = np.concatenate(
        [np.ones((P, P), np.float32), np.zeros((P, P), np.float32),
         np.full((P, P), 1.0 / D, np.float32), np.eye(P, dtype=np.float32)],
        axis=1).astype(bf16)

    # attention-output bias folds: c_o = Wo @ bv + b_out
    vecs = []
    for l in range(nl):
        bv = bsi[l, 2 * D:]
        cbv = bci[l, 2 * D:]
        vecs.append(_pack_cols(
            bsi[l, :D] * scale, bsi[l, D:2 * D],
            bci[l, :D] * scale, bci[l, D:2 * D],
            Wso[l] @ bv + bso[l], Wco[l] @ cbv + bco[l],
            f("b_ff2")[l],
            f("ln1_g")[l], f("ln1_b")[l],
            f("ln2_g")[l], f("ln2_b")[l],
            f("ln3_g")[l], f("ln3_b")[l],
            f("b_ff1")[l]))
    shared["vec"] = np.ascontiguousarray(np.stack(vecs))

    # combined causal mask over the [256 | 128] score strip:
    # cols 0:256 -> k-block0 vs q in [0,256); cols 256:384 -> k-block1
    # vs q in [128,256)
    kp = np.arange(P)[:, None]
    q = np.arange(SB)[None, :]
    m0 = (kp <= q).astype(np.float32)
    q2 = np.arange(P)[None, :]
    m1 = (kp <= q2).astype(np.float32)
    strip = np.concatenate([m0, m1], axis=1)
    shared["maskT"] = np.ascontiguousarray(
        np.concatenate([strip, strip], axis=1).astype(bf16))

    in_maps = []
    for c in range(NCORES):
        b0 = c * NB
        m = dict(shared)
        xc = np.ascontiguousarray(xpe[b0:b0 + NB].reshape(T, D).T)
        m["xpeB"] = xc.astype(bf16)
        m["memB"] = np.ascontiguousarray(
            enc[b0:b0 + NB].reshape(T, D).T.astype(bf16))
        in_maps.append(m)
    return in_maps


def gather(results):
    outs = []
    for r in results:
        o = np.asarray(r["outT"])          # [768, 512]
        outs.append(o.T.reshape(NB, SB, D))
    full = np.concatenate(outs, axis=0)     # [16, 256, 768]
    out = full.reshape(-1, 256, 256, 3)
    return np.ascontiguousarray(np.transpose(out, (0, 3, 1, 2)))


def run(inputs, trace=False, nl=L):
    nc = _get_nc(nl)
    in_maps = prepare(inputs, nl)
    res = run_bass_kernel_spmd(nc, in_maps, list(range(NCORES)),
                               trace=trace)
    return gather(res.results), res


def kernel(**inputs):
    out, _ = run(inputs)
    return out

